# revision 27
# baseline (speedup 1.0000x reference)
"""Trainium2 Bass kernel for nn_AttentionSACModel (sparse_attention).

Data-parallel across 8 NeuronCores: obs sharded along batch, params replicated.
On-device layout keeps batch on the matmul free dim (activations stored
feature-major / transposed); all host<->device layout changes happen in numpy.
"""
import sys
import os

if "/opt/trn_rl_repo" not in sys.path:
    sys.path.insert(0, "/opt/trn_rl_repo")

import numpy as np
import ml_dtypes
_bf16np = ml_dtypes.bfloat16

OWN_DIM = 7
INT_DIM = 7
N_INTR = 20
H = 3
D = 42
TOT = H * D            # 126
ATTN = 128
HID = 256
NOUT = 4
B = 32768
N_CORES = 8
BC = B // N_CORES      # 4096 rows per core
NB = 512               # batch tile (matmul free dim)
NT = BC // NB          # 8 tiles per core
ALPHA = 0.2            # leaky relu slope

_BUILT = {}


def _build_nc():
    import concourse.bacc as bacc
    import concourse.bass as bass
    import concourse.tile as tile
    from concourse import mybir

    f32 = mybir.dt.float32
    f32r = mybir.dt.float32r
    bf16 = mybir.dt.bfloat16
    AF = mybir.ActivationFunctionType
    ALU = mybir.AluOpType
    AX = mybir.AxisListType

    nc = bacc.Bacc()

    # ---- DRAM I/O ----
    xo_d = nc.dram_tensor("xo", [OWN_DIM, BC], bf16, kind="ExternalInput")
    xa_d = nc.dram_tensor("xa", [126, BC], bf16, kind="ExternalInput")       # interactors 0..17, row 7n+f
    xb_d = nc.dram_tensor("xb", [14, BC], bf16, kind="ExternalInput")        # interactors 18,19
    wia_d = nc.dram_tensor("wia", [126, 18 * 126], bf16, kind="ExternalInput")  # padded int-embed lhsT, n<18
    wib_d = nc.dram_tensor("wib", [14, 2 * 126], bf16, kind="ExternalInput")    # n=18,19
    wo_d = nc.dram_tensor("wo", [7, 126], bf16, kind="ExternalInput")
    wq_d = nc.dram_tensor("wqb", [126, 126], bf16, kind="ExternalInput")
    wk_d = nc.dram_tensor("wkb", [126, 126], bf16, kind="ExternalInput")
    wv_d = nc.dram_tensor("wvb", [126, 126], bf16, kind="ExternalInput")
    va_d = nc.dram_tensor("va32", [126, 32], bf16, kind="ExternalInput")
    ds_d = nc.dram_tensor("densel", [128, 3], bf16, kind="ExternalInput")
    eb_d = nc.dram_tensor("ebcsel", [128, 4 * 126], bf16, kind="ExternalInput")
    rb_d = nc.dram_tensor("rbc", [3, 126], f32r, kind="ExternalInput")
    wat_d = nc.dram_tensor("wat", [126, 128], f32r, kind="ExternalInput")
    wop_d = nc.dram_tensor("wop", [126, 128], bf16, kind="ExternalInput")
    wh1_d = nc.dram_tensor("wh1r", [128, 512], f32r, kind="ExternalInput")   # [p, kc*256+m]
    wh2_d = nc.dram_tensor("wh2r", [128, 512], f32r, kind="ExternalInput")
    wout_d = nc.dram_tensor("woutr", [128, 8], f32r, kind="ExternalInput")   # [p, kc*4+m]
    bown_d = nc.dram_tensor("bown", [126, 1], f32, kind="ExternalInput")
    bint_d = nc.dram_tensor("bint", [126, 1], f32, kind="ExternalInput")
    bat_d = nc.dram_tensor("bat", [128, 1], f32, kind="ExternalInput")
    bop_d = nc.dram_tensor("bop", [128, 1], f32, kind="ExternalInput")
    bh1_d = nc.dram_tensor("bh1", [128, 2], f32, kind="ExternalInput")
    bh2_d = nc.dram_tensor("bh2", [128, 2], f32, kind="ExternalInput")
    bout_d = nc.dram_tensor("bout", [4, 1], f32, kind="ExternalInput")
    out_d = nc.dram_tensor("outT", [NOUT, BC], f32, kind="ExternalOutput")

    with tile.TileContext(nc) as tc:
        with tc.tile_pool(name="const", bufs=1) as cst, \
             tc.tile_pool(name="px", bufs=2) as px, \
             tc.tile_pool(name="pemb", bufs=3) as pemb, \
             tc.tile_pool(name="peng", bufs=4) as peng, \
             tc.tile_pool(name="pE", bufs=10) as pE, \
             tc.tile_pool(name="pv", bufs=2) as pv, \
             tc.tile_pool(name="pp", bufs=2) as pp, \
             tc.tile_pool(name="pn", bufs=6) as pn, \
             tc.tile_pool(name="ph", bufs=2) as ph, \
             tc.tile_pool(name="pz", bufs=2, space="PSUM") as ppz, \
             tc.tile_pool(name="pk", bufs=2, space="PSUM") as ppk, \
             tc.tile_pool(name="sm", bufs=3, space="PSUM") as small, \
             tc.tile_pool(name="pd", bufs=1, space="PSUM") as ppd:

            # ---- load constants ----
            WiA = cst.tile([126, 18 * 126], bf16)
            WiB = cst.tile([14, 2 * 126], bf16)
            Wo = cst.tile([7, 126], bf16)
            Wq = cst.tile([126, 126], bf16)
            Wk = cst.tile([126, 126], bf16)
            Wv = cst.tile([126, 126], bf16)
            Va = cst.tile([126, 32], bf16)
            Ds = cst.tile([128, 3], bf16)
            Eb = cst.tile([128, 4 * 126], bf16)
            Rb = cst.tile([3, 126], f32r)
            Wat = cst.tile([126, 128], f32r)
            Wop = cst.tile([126, 128], bf16)
            WH1 = cst.tile([128, 512], f32r)
            WH2 = cst.tile([128, 512], f32r)
            WOUT = cst.tile([128, 8], f32r)
            Bown = cst.tile([126, 1], f32)
            Bint = cst.tile([126, 1], f32)
            Bat = cst.tile([128, 1], f32)
            Bop = cst.tile([128, 1], f32)
            BH1 = cst.tile([128, 2], f32)
            BH2 = cst.tile([128, 2], f32)
            Bout = cst.tile([4, 1], f32)
            for t_sb, t_dr in [(WiA, wia_d), (Wo, wo_d), (Bown, bown_d),
                               (Bint, bint_d), (WiB, wib_d), (Wk, wk_d),
                               (Wq, wq_d), (Wv, wv_d), (Va, va_d), (Ds, ds_d),
                               (Eb, eb_d), (Rb, rb_d), (Wat, wat_d), (Wop, wop_d),
                               (WH1, wh1_d), (WH2, wh2_d), (WOUT, wout_d),
                               (Bat, bat_d), (Bop, bop_d), (BH1, bh1_d),
                               (BH2, bh2_d), (Bout, bout_d)]:
                nc.scalar.dma_start(out=t_sb, in_=t_dr[:, :])

            with nc.allow_low_precision(reason="bf16/f32r intermediates; final accums are f32"):
                state = {}

                def load_x(t):
                    bs = t * NB
                    XO = px.tile([OWN_DIM, NB], bf16, tag="xo", name="XO")
                    XA = px.tile([126, NB], bf16, tag="xa", name="XA")
                    XB = px.tile([14, NB], bf16, tag="xb", name="XB")
                    nc.sync.dma_start(out=XO, in_=xo_d[:, bs:bs + NB])
                    nc.sync.dma_start(out=XA, in_=xa_d[:, bs:bs + NB])
                    nc.sync.dma_start(out=XB, in_=xb_d[:, bs:bs + NB])
                    state[t] = {"X": (XO, XA, XB)}

                def emit_den(t):
                    EGs = state[t]["EGs"]
                    PD = ppd.tile([128, NB], f32, tag="pd", name="PD")
                    for g in range(5):
                        nc.tensor.matmul(PD[0:3, :], Ds, EGs[g],
                                         start=(g == 0), stop=(g == 4))
                    RD = ph.tile([3, NB], f32r, tag="rd", name="RD")
                    nc.vector.reciprocal(RD, PD[0:3, :])
                    state[t]["RD"] = RD

                def emit_pr(t):
                    PR = small.tile([128, NB], f32, tag="sm", name="PR")
                    nc.tensor.matmul(PR[0:126, :], Rb, state[t]["RD"])
                    state[t]["PR"] = PR

                def merged(t, tb):
                    """ctx phase of tile t (may be None) interleaved with
                    embed/attention phase of tile tb (may be None)."""
                    st = state.get(t)
                    if st is not None:
                        EGs = st["EGs"]
                        VA = st["VA"]
                        PR = st["PR"]
                        TST = pp.tile([126, NB, N_INTR // 2], f32, tag="tst", name="TST")
                        PNs = []

                    if tb is not None:
                        XO, XA, XB = state[tb]["X"]
                        PO = small.tile([128, NB], f32, tag="sm", name="PO")
                        nc.tensor.matmul(PO[0:126, :], Wo, XO)
                        OWN = ph.tile([126, NB], bf16, tag="own", name="OWN", bufs=4)
                        nc.scalar.activation(OWN, PO[0:126, :], AF.Prelu, bias=Bown, alpha=ALPHA)
                        EGsb = []
                        VAb = pv.tile([126, N_INTR, NB], bf16, tag="va", name="VAb")
                        ZTs = {}
                        ENs = {}

                        def emit_z(n):
                            PZ = ppz.tile([126, NB], f32, tag="pz", name="PZ")
                            if n < 18:
                                nc.tensor.matmul(PZ, WiA[:, n * 126:(n + 1) * 126], XA)
                            else:
                                nc.tensor.matmul(PZ, WiB[:, (n - 18) * 126:(n - 17) * 126], XB)
                            ZT = pemb.tile([126, NB], bf16, tag="zt", name="ZT")
                            nc.scalar.activation(ZT, PZ, AF.Prelu, bias=Bint, alpha=ALPHA)
                            ZTs[n] = ZT

                        emit_z(0)
                        emit_z(1)
                        sc_pend = []

                        def emit_score(n, EN):
                            j = n % 4
                            if j == 0:
                                sc_pend.append(small.tile([128, NB], f32, tag="sm", name="PS"))
                            PSq = sc_pend[-1]
                            nc.tensor.matmul(PSq[32 * j:32 * (j + 1), :], Va, EN,
                                             tile_position=(0, 32 * j))
                            if j == 3:
                                EG = pE.tile([128, NB], bf16, tag="eg", name="EG")
                                nc.scalar.activation(EG, PSq, AF.Exp)
                                EGsb.append(EG)

                    for n in range(N_INTR):
                        if tb is not None:
                            ZT = ZTs.pop(n)
                            PK = ppk.tile([126, NB], f32, tag="pk", name="PK")
                            EN = peng.tile([126, NB], bf16, tag="en", name="EN")
                            nc.tensor.matmul(PK, Wk, ZT, start=True, stop=False)
                            nc.tensor.matmul(PK, Wq, OWN, start=False, stop=True)
                            nc.scalar.activation(EN, PK, AF.Tanh)

                            PV = small.tile([128, NB], f32, tag="sm", name="PV")
                            nc.tensor.matmul(PV[0:126, :], Wv, ZT)
                            nc.scalar.activation(VAb[:, n, :], PV[0:126, :], AF.Copy)

                            if n + 2 < N_INTR:
                                emit_z(n + 2)

                            j = n % 4
                            if j == 0:
                                PS = small.tile([128, NB], f32, tag="sm", name="PS")
                            nc.tensor.matmul(PS[32 * j:32 * (j + 1), :], Va, EN,
                                             tile_position=(0, 32 * j))
                            if j == 3:
                                EG = pE.tile([128, NB], bf16, tag="eg", name="EG")
                                nc.scalar.activation(EG, PS, AF.Exp)
                                EGsb.append(EG)

                        if st is not None and tb is None and n == 12:
                            CTXH0 = ph.tile([126, NB], f32, tag="ctxh", name="CTXH0")
                            nc.vector.tensor_reduce(CTXH0, TST[:, :, 0:5], axis=AX.X, op=ALU.add)
                            st["CTXH0"] = CTXH0
                        if st is not None:
                            g, j = n // 4, n % 4
                            PEb = small.tile([128, NB], f32, tag="sm", name="PEb")
                            nc.tensor.matmul(PEb[0:126, :], Eb[:, j * 126:(j + 1) * 126], EGs[g])
                            PN = pn.tile([126, NB], f32, tag="pn", name="PN")
                            nc.vector.tensor_tensor(out=PN, in0=PEb[0:126, :],
                                                    in1=VA[:, n, :], op=ALU.mult)
                            PNs.append(PN)
                            if n % 2 == 1:
                                nc.gpsimd.tensor_add(out=TST[:, :, n // 2],
                                                     in0=PNs[n - 1], in1=PNs[n])

                    if st is not None:
                        CTXU = ph.tile([126, NB], f32, tag="ctxu", name="CTXU")
                        if tb is None:
                            CTXH = st["CTXH0"]
                            CTXI = ph.tile([126, NB], f32, tag="ctxi", name="CTXI")
                            nc.vector.tensor_reduce(CTXI, TST[:, :, 5:10], axis=AX.X, op=ALU.add)
                            nc.vector.tensor_tensor(out=CTXU, in0=CTXH, in1=CTXI, op=ALU.add)
                        else:
                            nc.vector.tensor_reduce(CTXU, TST[:, :, :], axis=AX.X, op=ALU.add)
                        CTX = ph.tile([126, NB], f32r, tag="ctx", name="CTX")
                        nc.vector.tensor_tensor(out=CTX, in0=CTXU, in1=PR[0:126, :], op=ALU.mult)
                        st["CTX"] = CTX
                    if tb is not None:
                        state[tb].update({"OWN": OWN, "VA": VAb, "EGs": EGsb})

                def head_steps(t):
                    """head MLP + output for tile t, as interleavable steps"""
                    bs = t * NB
                    OWN = state[t]["OWN"]
                    h = {}

                    def s1():
                        PH1 = small.tile([128, NB], f32, tag="sm", name="PH1")
                        nc.tensor.matmul(PH1, Wat, state[t]["CTX"])
                        h["ATT"] = ph.tile([128, NB], f32r, tag="att", name="ATT")
                        nc.scalar.activation(h["ATT"], PH1, AF.Tanh, bias=Bat)

                    def s2():
                        PH2 = small.tile([128, NB], f32, tag="sm", name="PH2")
                        nc.tensor.matmul(PH2, Wop, OWN)
                        h["OWV"] = ph.tile([128, NB], f32r, tag="owv", name="OWV")
                        nc.scalar.activation(h["OWV"], PH2, AF.Tanh, bias=Bop)

                    def mk_h1(mh):
                        def s():
                            PHh = small.tile([128, NB], f32, tag="sm", name="PHh")
                            nc.tensor.matmul(PHh, WH1[:, mh * 128:(mh + 1) * 128], h["OWV"],
                                             start=True, stop=False)
                            nc.tensor.matmul(PHh, WH1[:, 256 + mh * 128:256 + (mh + 1) * 128],
                                             h["ATT"], start=False, stop=True)
                            h[f"H1{mh}"] = ph.tile([128, NB], f32r, tag=f"h1a{mh}", name="H1A")
                            nc.scalar.activation(h[f"H1{mh}"], PHh, AF.Prelu,
                                                 bias=BH1[:, mh:mh + 1], alpha=ALPHA)
                        return s

                    def mk_h2(mh):
                        def s():
                            PHh2 = small.tile([128, NB], f32, tag="sm", name="PHh2")
                            nc.tensor.matmul(PHh2, WH2[:, mh * 128:(mh + 1) * 128], h["H10"],
                                             start=True, stop=False)
                            nc.tensor.matmul(PHh2, WH2[:, 256 + mh * 128:256 + (mh + 1) * 128],
                                             h["H11"], start=False, stop=True)
                            h[f"H2{mh}"] = ph.tile([128, NB], f32r, tag=f"h2a{mh}", name="H2A")
                            nc.scalar.activation(h[f"H2{mh}"], PHh2, AF.Prelu,
                                                 bias=BH2[:, mh:mh + 1], alpha=ALPHA)
                        return s

                    def s7():
                        PO4 = small.tile([128, NB], f32, tag="sm", name="PO4")
                        nc.tensor.matmul(PO4[0:4, :], WOUT[:, 0:4], h["H20"], start=True, stop=False)
                        nc.tensor.matmul(PO4[0:4, :], WOUT[:, 4:8], h["H21"], start=False, stop=True)
                        OT = ph.tile([4, NB], f32, tag="ot", name="OT")
                        nc.scalar.activation(OT, PO4[0:4, :], AF.Identity, bias=Bout)
                        nc.sync.dma_start(out=out_d[:, bs:bs + NB], in_=OT)
                        del state[t]

                    return [s1, s2, mk_h1(0), mk_h1(1), mk_h2(0), mk_h2(1), s7]

                # 3-deep software pipeline over tiles; head steps of tile
                # t-2 are spread through merged(t-1, t) so the head chain's
                # ACT latencies hide behind dense PE work
                def run_head(t, den_t):
                    hs = head_steps(t)
                    hs[0]()
                    hs[1]()
                    if den_t is not None:
                        emit_den(den_t)
                    hs[2]()
                    hs[3]()
                    if den_t is not None:
                        emit_pr(den_t)
                    hs[4]()
                    hs[5]()
                    hs[6]()

                load_x(0)
                load_x(1)
                merged(None, 0)
                emit_den(0)
                emit_pr(0)
                for t in range(1, NT):
                    if t + 1 < NT:
                        load_x(t + 1)
                    merged(t - 1, t)
                    if t < NT - 1:
                        if t >= 2:
                            run_head(t - 2, t)
                        else:
                            emit_den(t)
                            emit_pr(t)
                    else:
                        run_head(t - 2, t)
                run_head(NT - 2, None)
                hs_last = head_steps(NT - 1)
                hs_last[1]()          # ownp: depends only on OWN, hide under ctx
                merged(NT - 1, None)
                hs_last[0]()
                for fn in hs_last[2:]:
                    fn()

    nc.compile()
    return nc


def _host_prep(inputs):
    """Build per-core input maps (numpy only)."""
    obs = np.ascontiguousarray(inputs["obs"], dtype=np.float32)
    w_own = np.asarray(inputs["w_own"], np.float32)
    w_int = np.asarray(inputs["w_int"], np.float32)
    wq = np.asarray(inputs["wq"], np.float32)
    wk = np.asarray(inputs["wk"], np.float32)
    wv = np.asarray(inputs["wv"], np.float32)
    v_att = np.asarray(inputs["v_att"], np.float32)
    w_attn = np.asarray(inputs["w_attn"], np.float32)
    w_ownp = np.asarray(inputs["w_ownp"], np.float32)
    w_h1 = np.asarray(inputs["w_h1"], np.float32)
    w_h2 = np.asarray(inputs["w_h2"], np.float32)
    w_out = np.asarray(inputs["w_out"], np.float32)

    def blockdiag(w):  # [H, D, D] -> [126, 126]
        out = np.zeros((TOT, TOT), np.float32)
        for h in range(H):
            out[h * D:(h + 1) * D, h * D:(h + 1) * D] = w[h]
        return out

    wia = np.zeros((126, 18 * 126), np.float32)
    for n in range(18):
        wia[7 * n:7 * n + 7, n * 126:(n + 1) * 126] = w_int
    wib = np.zeros((14, 2 * 126), np.float32)
    for n in range(2):
        wib[7 * n:7 * n + 7, n * 126:(n + 1) * 126] = w_int

    va32 = np.zeros((126, 32), np.float32)
    for h in range(H):
        va32[h * D:(h + 1) * D, h] = v_att[h]

    densel = np.zeros((128, 3), np.float32)
    for j in range(4):
        for h in range(H):
            densel[32 * j + h, h] = 1.0

    ebcsel = np.zeros((128, 4 * 126), np.float32)
    for j in range(4):
        for h in range(H):
            ebcsel[32 * j + h, j * 126 + h * D:(j * 126) + (h + 1) * D] = 1.0

    rbc = np.zeros((3, 126), np.float32)
    for h in range(H):
        rbc[h, h * D:(h + 1) * D] = 1.0

    wh1r = np.ascontiguousarray(
        w_h1.reshape(2, 128, HID).transpose(1, 0, 2).reshape(128, 512))
    wh2r = np.ascontiguousarray(
        w_h2.reshape(2, 128, HID).transpose(1, 0, 2).reshape(128, 512))
    woutr = np.ascontiguousarray(
        w_out.reshape(2, 128, NOUT).transpose(1, 0, 2).reshape(128, 8))

    params = {
        "wia": wia.astype(_bf16np), "wib": wib.astype(_bf16np), "wo": w_own.astype(_bf16np),
        "wqb": blockdiag(wq).astype(_bf16np), "wkb": blockdiag(wk).astype(_bf16np), "wvb": blockdiag(wv).astype(_bf16np),
        "va32": va32.astype(_bf16np), "densel": densel.astype(_bf16np), "ebcsel": ebcsel.astype(_bf16np), "rbc": rbc,
        "wat": w_attn, "wop": w_ownp.astype(_bf16np),
        "wh1r": wh1r, "wh2r": wh2r, "woutr": woutr,
        "bown": np.asarray(inputs["b_own"], np.float32).reshape(126, 1),
        "bint": np.asarray(inputs["b_int"], np.float32).reshape(126, 1),
        "bat": np.asarray(inputs["b_attn"], np.float32).reshape(128, 1),
        "bop": np.asarray(inputs["b_ownp"], np.float32).reshape(128, 1),
        "bh1": np.ascontiguousarray(
            np.asarray(inputs["b_h1"], np.float32).reshape(2, 128).T),
        "bh2": np.ascontiguousarray(
            np.asarray(inputs["b_h2"], np.float32).reshape(2, 128).T),
        "bout": np.asarray(inputs["b_out"], np.float32).reshape(4, 1),
    }

    in_maps = []
    for c in range(N_CORES):
        sl = obs[c * BC:(c + 1) * BC]                       # [BC, 147]
        xo = np.ascontiguousarray(sl[:, :OWN_DIM].T).astype(_bf16np)        # [7, BC]
        intr = sl[:, OWN_DIM:].reshape(BC, N_INTR, INT_DIM)  # [BC, 20, 7]
        intrT = intr.transpose(1, 2, 0)                     # [20, 7, BC]
        xa = np.ascontiguousarray(intrT[:18].reshape(126, BC)).astype(_bf16np)
        xb = np.ascontiguousarray(intrT[18:].reshape(14, BC)).astype(_bf16np)
        m = {"xo": xo, "xa": xa, "xb": xb}
        m.update(params)
        in_maps.append(m)
    return in_maps


def _get_nc():
    if "nc" not in _BUILT:
        _BUILT["nc"] = _build_nc()
    return _BUILT["nc"]


def run(inputs, trace=False):
    from concourse.bass_utils import run_bass_kernel_spmd
    nc = _get_nc()
    in_maps = _host_prep(inputs)
    res = run_bass_kernel_spmd(nc, in_maps, core_ids=list(range(N_CORES)),
                               trace=trace)
    outs = [res.results[c]["outT"] for c in range(N_CORES)]   # each [4, BC]
    full = np.concatenate(outs, axis=1).T                     # [B, 4]
    return np.ascontiguousarray(full, dtype=np.float32), res


def kernel(**inputs):
    out, _ = run(inputs, trace=False)
    return out


# revision 28
# speedup vs baseline: 1.0295x; 1.0295x over previous
"""Trainium2 Bass kernel for nn_AttentionSACModel (sparse_attention).

Data-parallel across 8 NeuronCores: obs sharded along batch, params replicated.
On-device layout keeps batch on the matmul free dim (activations stored
feature-major / transposed); all host<->device layout changes happen in numpy.
"""
import sys
import os

if "/opt/trn_rl_repo" not in sys.path:
    sys.path.insert(0, "/opt/trn_rl_repo")

import numpy as np
import ml_dtypes
_bf16np = ml_dtypes.bfloat16

OWN_DIM = 7
INT_DIM = 7
N_INTR = 20
H = 3
D = 42
TOT = H * D            # 126
ATTN = 128
HID = 256
NOUT = 4
B = 32768
N_CORES = 8
BC = B // N_CORES      # 4096 rows per core
NB = 512               # batch tile (matmul free dim)
NT = BC // NB          # 8 tiles per core
ALPHA = 0.2            # leaky relu slope

_BUILT = {}


def _build_nc():
    import concourse.bacc as bacc
    import concourse.bass as bass
    import concourse.tile as tile
    from concourse import mybir

    f32 = mybir.dt.float32
    f32r = mybir.dt.float32r
    bf16 = mybir.dt.bfloat16
    AF = mybir.ActivationFunctionType
    ALU = mybir.AluOpType
    AX = mybir.AxisListType

    nc = bacc.Bacc()

    # ---- DRAM I/O ----
    xo_d = nc.dram_tensor("xo", [OWN_DIM, BC], bf16, kind="ExternalInput")
    xa_d = nc.dram_tensor("xa", [126, BC], bf16, kind="ExternalInput")       # interactors 0..17, row 7n+f
    xb_d = nc.dram_tensor("xb", [14, BC], bf16, kind="ExternalInput")        # interactors 18,19
    wia_d = nc.dram_tensor("wia", [126, 18 * 126], bf16, kind="ExternalInput")  # padded int-embed lhsT, n<18
    wib_d = nc.dram_tensor("wib", [14, 2 * 126], bf16, kind="ExternalInput")    # n=18,19
    wo_d = nc.dram_tensor("wo", [7, 126], bf16, kind="ExternalInput")
    wq_d = nc.dram_tensor("wqb", [126, 126], bf16, kind="ExternalInput")
    wk_d = nc.dram_tensor("wkb", [126, 126], bf16, kind="ExternalInput")
    wv_d = nc.dram_tensor("wvb", [126, 126], bf16, kind="ExternalInput")
    va_d = nc.dram_tensor("va32", [126, 32], bf16, kind="ExternalInput")
    ds_d = nc.dram_tensor("densel", [128, 3], bf16, kind="ExternalInput")
    eb_d = nc.dram_tensor("ebcsel", [128, 4 * 126], bf16, kind="ExternalInput")
    rb_d = nc.dram_tensor("rbc", [3, 126], f32r, kind="ExternalInput")
    wat_d = nc.dram_tensor("wat", [126, 128], f32r, kind="ExternalInput")
    wop_d = nc.dram_tensor("wop", [126, 128], bf16, kind="ExternalInput")
    wh1_d = nc.dram_tensor("wh1r", [128, 512], f32r, kind="ExternalInput")   # [p, kc*256+m]
    wh2_d = nc.dram_tensor("wh2r", [128, 512], f32r, kind="ExternalInput")
    wout_d = nc.dram_tensor("woutr", [128, 8], f32r, kind="ExternalInput")   # [p, kc*4+m]
    bown_d = nc.dram_tensor("bown", [126, 1], f32, kind="ExternalInput")
    bint_d = nc.dram_tensor("bint", [126, 1], f32, kind="ExternalInput")
    bat_d = nc.dram_tensor("bat", [128, 1], f32, kind="ExternalInput")
    bop_d = nc.dram_tensor("bop", [128, 1], f32, kind="ExternalInput")
    bh1_d = nc.dram_tensor("bh1", [128, 2], f32, kind="ExternalInput")
    bh2_d = nc.dram_tensor("bh2", [128, 2], f32, kind="ExternalInput")
    bout_d = nc.dram_tensor("bout", [4, 1], f32, kind="ExternalInput")
    out_d = nc.dram_tensor("outT", [NOUT, BC], f32, kind="ExternalOutput")

    with tile.TileContext(nc) as tc:
        with tc.tile_pool(name="const", bufs=1) as cst, \
             tc.tile_pool(name="px", bufs=2) as px, \
             tc.tile_pool(name="pemb", bufs=3) as pemb, \
             tc.tile_pool(name="peng", bufs=4) as peng, \
             tc.tile_pool(name="pE", bufs=10) as pE, \
             tc.tile_pool(name="pv", bufs=2) as pv, \
             tc.tile_pool(name="pp", bufs=2) as pp, \
             tc.tile_pool(name="pn", bufs=6) as pn, \
             tc.tile_pool(name="ph", bufs=2) as ph, \
             tc.tile_pool(name="pz", bufs=2, space="PSUM") as ppz, \
             tc.tile_pool(name="pk", bufs=2, space="PSUM") as ppk, \
             tc.tile_pool(name="sm", bufs=3, space="PSUM") as small, \
             tc.tile_pool(name="pd", bufs=1, space="PSUM") as ppd:

            # ---- load constants ----
            WiA = cst.tile([126, 18 * 126], bf16)
            WiB = cst.tile([14, 2 * 126], bf16)
            Wo = cst.tile([7, 126], bf16)
            Wq = cst.tile([126, 126], bf16)
            Wk = cst.tile([126, 126], bf16)
            Wv = cst.tile([126, 126], bf16)
            Va = cst.tile([126, 32], bf16)
            Ds = cst.tile([128, 3], bf16)
            Eb = cst.tile([128, 4 * 126], bf16)
            Rb = cst.tile([3, 126], f32r)
            Wat = cst.tile([126, 128], f32r)
            Wop = cst.tile([126, 128], bf16)
            WH1 = cst.tile([128, 512], f32r)
            WH2 = cst.tile([128, 512], f32r)
            WOUT = cst.tile([128, 8], f32r)
            Bown = cst.tile([126, 1], f32)
            Bint = cst.tile([126, 1], f32)
            Bat = cst.tile([128, 1], f32)
            Bop = cst.tile([128, 1], f32)
            BH1 = cst.tile([128, 2], f32)
            BH2 = cst.tile([128, 2], f32)
            Bout = cst.tile([4, 1], f32)
            for t_sb, t_dr in [(WiA, wia_d), (Wo, wo_d), (Bown, bown_d),
                               (Bint, bint_d), (WiB, wib_d), (Wk, wk_d),
                               (Wq, wq_d), (Wv, wv_d), (Va, va_d), (Ds, ds_d),
                               (Eb, eb_d), (Rb, rb_d), (Wat, wat_d), (Wop, wop_d),
                               (WH1, wh1_d), (WH2, wh2_d), (WOUT, wout_d),
                               (Bat, bat_d), (Bop, bop_d), (BH1, bh1_d),
                               (BH2, bh2_d), (Bout, bout_d)]:
                nc.scalar.dma_start(out=t_sb, in_=t_dr[:, :])

            with nc.allow_low_precision(reason="bf16/f32r intermediates; final accums are f32"):
                state = {}

                def load_x(t):
                    bs = t * NB
                    XO = px.tile([OWN_DIM, NB], bf16, tag="xo", name="XO")
                    XA = px.tile([126, NB], bf16, tag="xa", name="XA")
                    XB = px.tile([14, NB], bf16, tag="xb", name="XB")
                    nc.sync.dma_start(out=XO, in_=xo_d[:, bs:bs + NB])
                    nc.sync.dma_start(out=XA, in_=xa_d[:, bs:bs + NB])
                    nc.sync.dma_start(out=XB, in_=xb_d[:, bs:bs + NB])
                    state[t] = {"X": (XO, XA, XB)}

                def merged(t, tb):
                    """ctx phase of tile t (may be None) interleaved with
                    embed/attention phase of tile tb (may be None)."""
                    st = state.get(t)
                    if st is not None:
                        EGs = st["EGs"]
                        VA = st["VA"]
                        PD = ppd.tile([128, NB], f32, tag="pd", name="PD")
                        for g in range(5):
                            nc.tensor.matmul(PD[0:3, :], Ds, EGs[g],
                                             start=(g == 0), stop=(g == 4))
                        RD = ph.tile([3, NB], f32r, tag="rd", name="RD")
                        nc.vector.reciprocal(RD, PD[0:3, :])
                        PR = small.tile([128, NB], f32, tag="sm", name="PR")
                        nc.tensor.matmul(PR[0:126, :], Rb, RD)
                        TST = pp.tile([126, NB, N_INTR // 2], f32, tag="tst", name="TST")
                        PNs = []

                    if tb is not None:
                        XO, XA, XB = state[tb]["X"]
                        PO = small.tile([128, NB], f32, tag="sm", name="PO")
                        nc.tensor.matmul(PO[0:126, :], Wo, XO)
                        OWN = ph.tile([126, NB], bf16, tag="own", name="OWN", bufs=4)
                        nc.scalar.activation(OWN, PO[0:126, :], AF.Prelu, bias=Bown, alpha=ALPHA)
                        EGsb = []
                        VAb = pv.tile([126, N_INTR, NB], bf16, tag="va", name="VAb")
                        ZTs = {}
                        ENs = {}

                        def emit_z(n):
                            PZ = ppz.tile([126, NB], f32, tag="pz", name="PZ")
                            if n < 18:
                                nc.tensor.matmul(PZ, WiA[:, n * 126:(n + 1) * 126], XA)
                            else:
                                nc.tensor.matmul(PZ, WiB[:, (n - 18) * 126:(n - 17) * 126], XB)
                            ZT = pemb.tile([126, NB], bf16, tag="zt", name="ZT")
                            nc.scalar.activation(ZT, PZ, AF.Prelu, bias=Bint, alpha=ALPHA)
                            ZTs[n] = ZT

                        emit_z(0)
                        emit_z(1)
                        sc_pend = []

                        def emit_score(n, EN):
                            j = n % 4
                            if j == 0:
                                sc_pend.append(small.tile([128, NB], f32, tag="sm", name="PS"))
                            PSq = sc_pend[-1]
                            nc.tensor.matmul(PSq[32 * j:32 * (j + 1), :], Va, EN,
                                             tile_position=(0, 32 * j))
                            if j == 3:
                                EG = pE.tile([128, NB], bf16, tag="eg", name="EG")
                                nc.scalar.activation(EG, PSq, AF.Exp)
                                EGsb.append(EG)

                    for n in range(N_INTR):
                        if tb is not None:
                            ZT = ZTs.pop(n)
                            PK = ppk.tile([126, NB], f32, tag="pk", name="PK")
                            EN = peng.tile([126, NB], bf16, tag="en", name="EN")
                            nc.tensor.matmul(PK, Wk, ZT, start=True, stop=False)
                            nc.tensor.matmul(PK, Wq, OWN, start=False, stop=True)
                            nc.scalar.activation(EN, PK, AF.Tanh)

                            PV = small.tile([128, NB], f32, tag="sm", name="PV")
                            nc.tensor.matmul(PV[0:126, :], Wv, ZT)
                            nc.scalar.activation(VAb[:, n, :], PV[0:126, :], AF.Copy)

                            if n + 2 < N_INTR:
                                emit_z(n + 2)

                            j = n % 4
                            if j == 0:
                                PS = small.tile([128, NB], f32, tag="sm", name="PS")
                            nc.tensor.matmul(PS[32 * j:32 * (j + 1), :], Va, EN,
                                             tile_position=(0, 32 * j))
                            if j == 3:
                                EG = pE.tile([128, NB], bf16, tag="eg", name="EG")
                                nc.scalar.activation(EG, PS, AF.Exp)
                                EGsb.append(EG)

                        if st is not None and tb is None and n == 12:
                            CTXH0 = ph.tile([126, NB], f32, tag="ctxh", name="CTXH0")
                            nc.vector.tensor_reduce(CTXH0, TST[:, :, 0:5], axis=AX.X, op=ALU.add)
                            st["CTXH0"] = CTXH0
                        if st is not None:
                            g, j = n // 4, n % 4
                            PEb = small.tile([128, NB], f32, tag="sm", name="PEb")
                            nc.tensor.matmul(PEb[0:126, :], Eb[:, j * 126:(j + 1) * 126], EGs[g])
                            PN = pn.tile([126, NB], f32, tag="pn", name="PN")
                            nc.vector.tensor_tensor(out=PN, in0=PEb[0:126, :],
                                                    in1=VA[:, n, :], op=ALU.mult)
                            PNs.append(PN)
                            if n % 2 == 1:
                                nc.gpsimd.tensor_add(out=TST[:, :, n // 2],
                                                     in0=PNs[n - 1], in1=PNs[n])

                    if st is not None:
                        CTXU = ph.tile([126, NB], f32, tag="ctxu", name="CTXU")
                        if tb is None:
                            CTXH = st["CTXH0"]
                            CTXI = ph.tile([126, NB], f32, tag="ctxi", name="CTXI")
                            nc.vector.tensor_reduce(CTXI, TST[:, :, 5:10], axis=AX.X, op=ALU.add)
                            nc.vector.tensor_tensor(out=CTXU, in0=CTXH, in1=CTXI, op=ALU.add)
                        else:
                            nc.vector.tensor_reduce(CTXU, TST[:, :, :], axis=AX.X, op=ALU.add)
                        CTX = ph.tile([126, NB], f32r, tag="ctx", name="CTX")
                        nc.vector.tensor_tensor(out=CTX, in0=CTXU, in1=PR[0:126, :], op=ALU.mult)
                        st["CTX"] = CTX
                    if tb is not None:
                        state[tb].update({"OWN": OWN, "VA": VAb, "EGs": EGsb})

                def head_steps(t):
                    """head MLP + output for tile t, as interleavable steps"""
                    bs = t * NB
                    OWN = state[t]["OWN"]
                    h = {}

                    def s1():
                        PH1 = small.tile([128, NB], f32, tag="sm", name="PH1")
                        nc.tensor.matmul(PH1, Wat, state[t]["CTX"])
                        h["ATT"] = ph.tile([128, NB], f32r, tag="att", name="ATT")
                        nc.scalar.activation(h["ATT"], PH1, AF.Tanh, bias=Bat)

                    def s2():
                        PH2 = small.tile([128, NB], f32, tag="sm", name="PH2")
                        nc.tensor.matmul(PH2, Wop, OWN)
                        h["OWV"] = ph.tile([128, NB], f32r, tag="owv", name="OWV")
                        nc.scalar.activation(h["OWV"], PH2, AF.Tanh, bias=Bop)

                    def mk_h1(mh):
                        def s():
                            PHh = small.tile([128, NB], f32, tag="sm", name="PHh")
                            nc.tensor.matmul(PHh, WH1[:, mh * 128:(mh + 1) * 128], h["OWV"],
                                             start=True, stop=False)
                            nc.tensor.matmul(PHh, WH1[:, 256 + mh * 128:256 + (mh + 1) * 128],
                                             h["ATT"], start=False, stop=True)
                            h[f"H1{mh}"] = ph.tile([128, NB], f32r, tag=f"h1a{mh}", name="H1A")
                            nc.scalar.activation(h[f"H1{mh}"], PHh, AF.Prelu,
                                                 bias=BH1[:, mh:mh + 1], alpha=ALPHA)
                        return s

                    def mk_h2(mh):
                        def s():
                            PHh2 = small.tile([128, NB], f32, tag="sm", name="PHh2")
                            nc.tensor.matmul(PHh2, WH2[:, mh * 128:(mh + 1) * 128], h["H10"],
                                             start=True, stop=False)
                            nc.tensor.matmul(PHh2, WH2[:, 256 + mh * 128:256 + (mh + 1) * 128],
                                             h["H11"], start=False, stop=True)
                            h[f"H2{mh}"] = ph.tile([128, NB], f32r, tag=f"h2a{mh}", name="H2A")
                            nc.scalar.activation(h[f"H2{mh}"], PHh2, AF.Prelu,
                                                 bias=BH2[:, mh:mh + 1], alpha=ALPHA)
                        return s

                    def s7():
                        PO4 = small.tile([128, NB], f32, tag="sm", name="PO4")
                        nc.tensor.matmul(PO4[0:4, :], WOUT[:, 0:4], h["H20"], start=True, stop=False)
                        nc.tensor.matmul(PO4[0:4, :], WOUT[:, 4:8], h["H21"], start=False, stop=True)
                        OT = ph.tile([4, NB], f32, tag="ot", name="OT")
                        nc.scalar.activation(OT, PO4[0:4, :], AF.Identity, bias=Bout)
                        nc.sync.dma_start(out=out_d[:, bs:bs + NB], in_=OT)
                        del state[t]

                    return [s1, s2, mk_h1(0), mk_h1(1), mk_h2(0), mk_h2(1), s7]

                # 3-deep software pipeline over tiles; head steps of tile
                # t-2 are spread through merged(t-1, t) so the head chain's
                # ACT latencies hide behind dense PE work
                load_x(0)
                load_x(1)
                merged(None, 0)
                for t in range(1, NT):
                    if t + 1 < NT:
                        load_x(t + 1)
                    merged(t - 1, t)
                    if t >= 2:
                        for fn in head_steps(t - 2):
                            fn()
                for fn in head_steps(NT - 2):
                    fn()
                hs_last = head_steps(NT - 1)
                hs_last[1]()          # ownp: depends only on OWN, hide under ctx
                merged(NT - 1, None)
                hs_last[0]()
                for fn in hs_last[2:]:
                    fn()

    nc.compile()
    return nc


def _host_prep(inputs):
    """Build per-core input maps (numpy only)."""
    obs = np.ascontiguousarray(inputs["obs"], dtype=np.float32)
    w_own = np.asarray(inputs["w_own"], np.float32)
    w_int = np.asarray(inputs["w_int"], np.float32)
    wq = np.asarray(inputs["wq"], np.float32)
    wk = np.asarray(inputs["wk"], np.float32)
    wv = np.asarray(inputs["wv"], np.float32)
    v_att = np.asarray(inputs["v_att"], np.float32)
    w_attn = np.asarray(inputs["w_attn"], np.float32)
    w_ownp = np.asarray(inputs["w_ownp"], np.float32)
    w_h1 = np.asarray(inputs["w_h1"], np.float32)
    w_h2 = np.asarray(inputs["w_h2"], np.float32)
    w_out = np.asarray(inputs["w_out"], np.float32)

    def blockdiag(w):  # [H, D, D] -> [126, 126]
        out = np.zeros((TOT, TOT), np.float32)
        for h in range(H):
            out[h * D:(h + 1) * D, h * D:(h + 1) * D] = w[h]
        return out

    wia = np.zeros((126, 18 * 126), np.float32)
    for n in range(18):
        wia[7 * n:7 * n + 7, n * 126:(n + 1) * 126] = w_int
    wib = np.zeros((14, 2 * 126), np.float32)
    for n in range(2):
        wib[7 * n:7 * n + 7, n * 126:(n + 1) * 126] = w_int

    va32 = np.zeros((126, 32), np.float32)
    for h in range(H):
        va32[h * D:(h + 1) * D, h] = v_att[h]

    densel = np.zeros((128, 3), np.float32)
    for j in range(4):
        for h in range(H):
            densel[32 * j + h, h] = 1.0

    ebcsel = np.zeros((128, 4 * 126), np.float32)
    for j in range(4):
        for h in range(H):
            ebcsel[32 * j + h, j * 126 + h * D:(j * 126) + (h + 1) * D] = 1.0

    rbc = np.zeros((3, 126), np.float32)
    for h in range(H):
        rbc[h, h * D:(h + 1) * D] = 1.0

    wh1r = np.ascontiguousarray(
        w_h1.reshape(2, 128, HID).transpose(1, 0, 2).reshape(128, 512))
    wh2r = np.ascontiguousarray(
        w_h2.reshape(2, 128, HID).transpose(1, 0, 2).reshape(128, 512))
    woutr = np.ascontiguousarray(
        w_out.reshape(2, 128, NOUT).transpose(1, 0, 2).reshape(128, 8))

    params = {
        "wia": wia.astype(_bf16np), "wib": wib.astype(_bf16np), "wo": w_own.astype(_bf16np),
        "wqb": blockdiag(wq).astype(_bf16np), "wkb": blockdiag(wk).astype(_bf16np), "wvb": blockdiag(wv).astype(_bf16np),
        "va32": va32.astype(_bf16np), "densel": densel.astype(_bf16np), "ebcsel": ebcsel.astype(_bf16np), "rbc": rbc,
        "wat": w_attn, "wop": w_ownp.astype(_bf16np),
        "wh1r": wh1r, "wh2r": wh2r, "woutr": woutr,
        "bown": np.asarray(inputs["b_own"], np.float32).reshape(126, 1),
        "bint": np.asarray(inputs["b_int"], np.float32).reshape(126, 1),
        "bat": np.asarray(inputs["b_attn"], np.float32).reshape(128, 1),
        "bop": np.asarray(inputs["b_ownp"], np.float32).reshape(128, 1),
        "bh1": np.ascontiguousarray(
            np.asarray(inputs["b_h1"], np.float32).reshape(2, 128).T),
        "bh2": np.ascontiguousarray(
            np.asarray(inputs["b_h2"], np.float32).reshape(2, 128).T),
        "bout": np.asarray(inputs["b_out"], np.float32).reshape(4, 1),
    }

    in_maps = []
    for c in range(N_CORES):
        sl = obs[c * BC:(c + 1) * BC]                       # [BC, 147]
        xo = np.ascontiguousarray(sl[:, :OWN_DIM].T).astype(_bf16np)        # [7, BC]
        intr = sl[:, OWN_DIM:].reshape(BC, N_INTR, INT_DIM)  # [BC, 20, 7]
        intrT = intr.transpose(1, 2, 0)                     # [20, 7, BC]
        xa = np.ascontiguousarray(intrT[:18].reshape(126, BC)).astype(_bf16np)
        xb = np.ascontiguousarray(intrT[18:].reshape(14, BC)).astype(_bf16np)
        m = {"xo": xo, "xa": xa, "xb": xb}
        m.update(params)
        in_maps.append(m)
    return in_maps


def _get_nc():
    if "nc" not in _BUILT:
        _BUILT["nc"] = _build_nc()
    return _BUILT["nc"]


def run(inputs, trace=False):
    from concourse.bass_utils import run_bass_kernel_spmd
    nc = _get_nc()
    in_maps = _host_prep(inputs)
    res = run_bass_kernel_spmd(nc, in_maps, core_ids=list(range(N_CORES)),
                               trace=trace)
    outs = [res.results[c]["outT"] for c in range(N_CORES)]   # each [4, BC]
    full = np.concatenate(outs, axis=1).T                     # [B, 4]
    return np.ascontiguousarray(full, dtype=np.float32), res


def kernel(**inputs):
    out, _ = run(inputs, trace=False)
    return out


# revision 29
# speedup vs baseline: 1.0409x; 1.0110x over previous
"""Trainium2 Bass kernel for nn_AttentionSACModel (sparse_attention).

Data-parallel across 8 NeuronCores: obs sharded along batch, params replicated.
On-device layout keeps batch on the matmul free dim (activations stored
feature-major / transposed); all host<->device layout changes happen in numpy.
"""
import sys
import os

if "/opt/trn_rl_repo" not in sys.path:
    sys.path.insert(0, "/opt/trn_rl_repo")

import numpy as np
import ml_dtypes
_bf16np = ml_dtypes.bfloat16

OWN_DIM = 7
INT_DIM = 7
N_INTR = 20
H = 3
D = 42
TOT = H * D            # 126
ATTN = 128
HID = 256
NOUT = 4
B = 32768
N_CORES = 8
BC = B // N_CORES      # 4096 rows per core
NB = 512               # batch tile (matmul free dim)
NT = BC // NB          # 8 tiles per core
ALPHA = 0.2            # leaky relu slope

_BUILT = {}


def _build_nc():
    import concourse.bacc as bacc
    import concourse.bass as bass
    import concourse.tile as tile
    from concourse import mybir

    f32 = mybir.dt.float32
    f32r = mybir.dt.float32r
    bf16 = mybir.dt.bfloat16
    AF = mybir.ActivationFunctionType
    ALU = mybir.AluOpType
    AX = mybir.AxisListType

    nc = bacc.Bacc()

    # ---- DRAM I/O ----
    xo_d = nc.dram_tensor("xo", [OWN_DIM, BC], bf16, kind="ExternalInput")
    xa_d = nc.dram_tensor("xa", [126, BC], bf16, kind="ExternalInput")       # interactors 0..17, row 7n+f
    xb_d = nc.dram_tensor("xb", [14, BC], bf16, kind="ExternalInput")        # interactors 18,19
    wia_d = nc.dram_tensor("wia", [126, 18 * 126], bf16, kind="ExternalInput")  # padded int-embed lhsT, n<18
    wib_d = nc.dram_tensor("wib", [14, 2 * 126], bf16, kind="ExternalInput")    # n=18,19
    wo_d = nc.dram_tensor("wo", [7, 126], bf16, kind="ExternalInput")
    wq_d = nc.dram_tensor("wqb", [126, 126], bf16, kind="ExternalInput")
    wk_d = nc.dram_tensor("wkb", [126, 126], bf16, kind="ExternalInput")
    wv_d = nc.dram_tensor("wvb", [126, 126], bf16, kind="ExternalInput")
    va_d = nc.dram_tensor("va32", [126, 32], bf16, kind="ExternalInput")
    ds_d = nc.dram_tensor("densel", [128, 3], bf16, kind="ExternalInput")
    eb_d = nc.dram_tensor("ebcsel", [128, 4 * 126], bf16, kind="ExternalInput")
    rb_d = nc.dram_tensor("rbc", [3, 126], f32r, kind="ExternalInput")
    wat_d = nc.dram_tensor("wat", [126, 128], f32r, kind="ExternalInput")
    wop_d = nc.dram_tensor("wop", [126, 128], bf16, kind="ExternalInput")
    wh1_d = nc.dram_tensor("wh1r", [128, 512], f32r, kind="ExternalInput")   # [p, kc*256+m]
    wh2_d = nc.dram_tensor("wh2r", [128, 512], f32r, kind="ExternalInput")
    wout_d = nc.dram_tensor("woutr", [128, 8], f32r, kind="ExternalInput")   # [p, kc*4+m]
    bown_d = nc.dram_tensor("bown", [126, 1], f32, kind="ExternalInput")
    bint_d = nc.dram_tensor("bint", [126, 1], f32, kind="ExternalInput")
    bat_d = nc.dram_tensor("bat", [128, 1], f32, kind="ExternalInput")
    bop_d = nc.dram_tensor("bop", [128, 1], f32, kind="ExternalInput")
    bh1_d = nc.dram_tensor("bh1", [128, 2], f32, kind="ExternalInput")
    bh2_d = nc.dram_tensor("bh2", [128, 2], f32, kind="ExternalInput")
    bout_d = nc.dram_tensor("bout", [4, 1], f32, kind="ExternalInput")
    out_d = nc.dram_tensor("outT", [NOUT, BC], f32, kind="ExternalOutput")

    with tile.TileContext(nc) as tc:
        with tc.tile_pool(name="const", bufs=1) as cst, \
             tc.tile_pool(name="px", bufs=2) as px, \
             tc.tile_pool(name="pemb", bufs=3) as pemb, \
             tc.tile_pool(name="peng", bufs=4) as peng, \
             tc.tile_pool(name="pE", bufs=10) as pE, \
             tc.tile_pool(name="pv", bufs=2) as pv, \
             tc.tile_pool(name="pp", bufs=2) as pp, \
             tc.tile_pool(name="pn", bufs=8) as pn, \
             tc.tile_pool(name="ph", bufs=2) as ph, \
             tc.tile_pool(name="pz", bufs=2, space="PSUM") as ppz, \
             tc.tile_pool(name="pk", bufs=2, space="PSUM") as ppk, \
             tc.tile_pool(name="sm", bufs=3, space="PSUM") as small, \
             tc.tile_pool(name="pd", bufs=1, space="PSUM") as ppd:

            # ---- load constants ----
            WiA = cst.tile([126, 18 * 126], bf16)
            WiB = cst.tile([14, 2 * 126], bf16)
            Wo = cst.tile([7, 126], bf16)
            Wq = cst.tile([126, 126], bf16)
            Wk = cst.tile([126, 126], bf16)
            Wv = cst.tile([126, 126], bf16)
            Va = cst.tile([126, 32], bf16)
            Ds = cst.tile([128, 3], bf16)
            Eb = cst.tile([128, 4 * 126], bf16)
            Rb = cst.tile([3, 126], f32r)
            Wat = cst.tile([126, 128], f32r)
            Wop = cst.tile([126, 128], bf16)
            WH1 = cst.tile([128, 512], f32r)
            WH2 = cst.tile([128, 512], f32r)
            WOUT = cst.tile([128, 8], f32r)
            Bown = cst.tile([126, 1], f32)
            Bint = cst.tile([126, 1], f32)
            Bat = cst.tile([128, 1], f32)
            Bop = cst.tile([128, 1], f32)
            BH1 = cst.tile([128, 2], f32)
            BH2 = cst.tile([128, 2], f32)
            Bout = cst.tile([4, 1], f32)
            for t_sb, t_dr in [(WiA, wia_d), (Wo, wo_d), (Bown, bown_d),
                               (Bint, bint_d), (WiB, wib_d), (Wk, wk_d),
                               (Wq, wq_d), (Wv, wv_d), (Va, va_d)]:
                nc.sync.dma_start(out=t_sb, in_=t_dr[:, :])

            def load_late_consts():
                for t_sb, t_dr in [(Ds, ds_d), (Eb, eb_d), (Rb, rb_d),
                                   (Wat, wat_d), (Wop, wop_d), (WH1, wh1_d),
                                   (WH2, wh2_d), (WOUT, wout_d), (Bat, bat_d),
                                   (Bop, bop_d), (BH1, bh1_d), (BH2, bh2_d),
                                   (Bout, bout_d)]:
                    nc.scalar.dma_start(out=t_sb, in_=t_dr[:, :])

            with nc.allow_low_precision(reason="bf16/f32r intermediates; final accums are f32"):
                state = {}

                def load_x(t):
                    bs = t * NB
                    XO = px.tile([OWN_DIM, NB], bf16, tag="xo", name="XO")
                    XA = px.tile([126, NB], bf16, tag="xa", name="XA")
                    XB = px.tile([14, NB], bf16, tag="xb", name="XB")
                    nc.sync.dma_start(out=XO, in_=xo_d[:, bs:bs + NB])
                    nc.sync.dma_start(out=XA, in_=xa_d[:, bs:bs + NB])
                    nc.sync.dma_start(out=XB, in_=xb_d[:, bs:bs + NB])
                    state[t] = {"X": (XO, XA, XB)}

                def merged(t, tb):
                    """ctx phase of tile t (may be None) interleaved with
                    embed/attention phase of tile tb (may be None)."""
                    st = state.get(t)
                    if st is not None:
                        EGs = st["EGs"]
                        VA = st["VA"]
                        PD = ppd.tile([128, NB], f32, tag="pd", name="PD")
                        for g in range(5):
                            nc.tensor.matmul(PD[0:3, :], Ds, EGs[g],
                                             start=(g == 0), stop=(g == 4))
                        RD = ph.tile([3, NB], f32r, tag="rd", name="RD")
                        nc.vector.reciprocal(RD, PD[0:3, :])
                        PR = small.tile([128, NB], f32, tag="sm", name="PR")
                        nc.tensor.matmul(PR[0:126, :], Rb, RD)
                        TST = pp.tile([126, NB, N_INTR // 2], f32, tag="tst", name="TST")
                        PNs = []

                    if tb is not None:
                        XO, XA, XB = state[tb]["X"]
                        PO = small.tile([128, NB], f32, tag="sm", name="PO")
                        nc.tensor.matmul(PO[0:126, :], Wo, XO)
                        OWN = ph.tile([126, NB], bf16, tag="own", name="OWN", bufs=4)
                        nc.scalar.activation(OWN, PO[0:126, :], AF.Prelu, bias=Bown, alpha=ALPHA)
                        EGsb = []
                        VAb = pv.tile([126, N_INTR, NB], bf16, tag="va", name="VAb")
                        ZTs = {}
                        ENs = {}

                        def emit_z(n):
                            PZ = ppz.tile([126, NB], f32, tag="pz", name="PZ")
                            if n < 18:
                                nc.tensor.matmul(PZ, WiA[:, n * 126:(n + 1) * 126], XA)
                            else:
                                nc.tensor.matmul(PZ, WiB[:, (n - 18) * 126:(n - 17) * 126], XB)
                            ZT = pemb.tile([126, NB], bf16, tag="zt", name="ZT")
                            nc.scalar.activation(ZT, PZ, AF.Prelu, bias=Bint, alpha=ALPHA)
                            ZTs[n] = ZT

                        emit_z(0)
                        emit_z(1)
                        sc_pend = []

                        def emit_score(n, EN):
                            j = n % 4
                            if j == 0:
                                sc_pend.append(small.tile([128, NB], f32, tag="sm", name="PS"))
                            PSq = sc_pend[-1]
                            nc.tensor.matmul(PSq[32 * j:32 * (j + 1), :], Va, EN,
                                             tile_position=(0, 32 * j))
                            if j == 3:
                                EG = pE.tile([128, NB], bf16, tag="eg", name="EG")
                                nc.scalar.activation(EG, PSq, AF.Exp)
                                EGsb.append(EG)

                    for n in range(N_INTR):
                        if tb is not None:
                            ZT = ZTs.pop(n)
                            PK = ppk.tile([126, NB], f32, tag="pk", name="PK")
                            EN = peng.tile([126, NB], bf16, tag="en", name="EN")
                            nc.tensor.matmul(PK, Wk, ZT, start=True, stop=False)
                            nc.tensor.matmul(PK, Wq, OWN, start=False, stop=True)
                            nc.scalar.activation(EN, PK, AF.Tanh)

                            PV = small.tile([128, NB], f32, tag="sm", name="PV")
                            nc.tensor.matmul(PV[0:126, :], Wv, ZT)
                            nc.scalar.activation(VAb[:, n, :], PV[0:126, :], AF.Copy)

                            if n + 2 < N_INTR:
                                emit_z(n + 2)

                            j = n % 4
                            if j == 0:
                                PS = small.tile([128, NB], f32, tag="sm", name="PS")
                            nc.tensor.matmul(PS[32 * j:32 * (j + 1), :], Va, EN,
                                             tile_position=(0, 32 * j))
                            if j == 3:
                                EG = pE.tile([128, NB], bf16, tag="eg", name="EG")
                                nc.scalar.activation(EG, PS, AF.Exp)
                                EGsb.append(EG)

                        if st is not None and tb is None and n == 12:
                            CTXH0 = ph.tile([126, NB], f32, tag="ctxh", name="CTXH0")
                            nc.vector.tensor_reduce(CTXH0, TST[:, :, 0:5], axis=AX.X, op=ALU.add)
                            st["CTXH0"] = CTXH0
                        if st is not None:
                            g, j = n // 4, n % 4
                            PEb = small.tile([128, NB], f32, tag="sm", name="PEb")
                            nc.tensor.matmul(PEb[0:126, :], Eb[:, j * 126:(j + 1) * 126], EGs[g])
                            PN = pn.tile([126, NB], f32, tag="pn", name="PN")
                            nc.vector.tensor_tensor(out=PN, in0=PEb[0:126, :],
                                                    in1=VA[:, n, :], op=ALU.mult)
                            PNs.append(PN)
                            if n % 2 == 1:
                                nc.gpsimd.tensor_add(out=TST[:, :, n // 2],
                                                     in0=PNs[n - 1], in1=PNs[n])

                    if st is not None:
                        CTXU = ph.tile([126, NB], f32, tag="ctxu", name="CTXU")
                        if tb is None:
                            CTXH = st["CTXH0"]
                            CTXI = ph.tile([126, NB], f32, tag="ctxi", name="CTXI")
                            nc.vector.tensor_reduce(CTXI, TST[:, :, 5:10], axis=AX.X, op=ALU.add)
                            nc.vector.tensor_tensor(out=CTXU, in0=CTXH, in1=CTXI, op=ALU.add)
                        else:
                            nc.vector.tensor_reduce(CTXU, TST[:, :, :], axis=AX.X, op=ALU.add)
                        CTX = ph.tile([126, NB], f32r, tag="ctx", name="CTX")
                        nc.vector.tensor_tensor(out=CTX, in0=CTXU, in1=PR[0:126, :], op=ALU.mult)
                        st["CTX"] = CTX
                    if tb is not None:
                        state[tb].update({"OWN": OWN, "VA": VAb, "EGs": EGsb})

                def head_steps(t):
                    """head MLP + output for tile t, as interleavable steps"""
                    bs = t * NB
                    OWN = state[t]["OWN"]
                    h = {}

                    def s1():
                        PH1 = small.tile([128, NB], f32, tag="sm", name="PH1")
                        nc.tensor.matmul(PH1, Wat, state[t]["CTX"])
                        h["ATT"] = ph.tile([128, NB], f32r, tag="att", name="ATT")
                        nc.scalar.activation(h["ATT"], PH1, AF.Tanh, bias=Bat)

                    def s2():
                        PH2 = small.tile([128, NB], f32, tag="sm", name="PH2")
                        nc.tensor.matmul(PH2, Wop, OWN)
                        h["OWV"] = ph.tile([128, NB], f32r, tag="owv", name="OWV")
                        nc.scalar.activation(h["OWV"], PH2, AF.Tanh, bias=Bop)

                    def mk_h1(mh):
                        def s():
                            PHh = small.tile([128, NB], f32, tag="sm", name="PHh")
                            nc.tensor.matmul(PHh, WH1[:, mh * 128:(mh + 1) * 128], h["OWV"],
                                             start=True, stop=False)
                            nc.tensor.matmul(PHh, WH1[:, 256 + mh * 128:256 + (mh + 1) * 128],
                                             h["ATT"], start=False, stop=True)
                            h[f"H1{mh}"] = ph.tile([128, NB], f32r, tag=f"h1a{mh}", name="H1A")
                            nc.scalar.activation(h[f"H1{mh}"], PHh, AF.Prelu,
                                                 bias=BH1[:, mh:mh + 1], alpha=ALPHA)
                        return s

                    def mk_h2(mh):
                        def s():
                            PHh2 = small.tile([128, NB], f32, tag="sm", name="PHh2")
                            nc.tensor.matmul(PHh2, WH2[:, mh * 128:(mh + 1) * 128], h["H10"],
                                             start=True, stop=False)
                            nc.tensor.matmul(PHh2, WH2[:, 256 + mh * 128:256 + (mh + 1) * 128],
                                             h["H11"], start=False, stop=True)
                            h[f"H2{mh}"] = ph.tile([128, NB], f32r, tag=f"h2a{mh}", name="H2A")
                            nc.scalar.activation(h[f"H2{mh}"], PHh2, AF.Prelu,
                                                 bias=BH2[:, mh:mh + 1], alpha=ALPHA)
                        return s

                    def s7():
                        PO4 = small.tile([128, NB], f32, tag="sm", name="PO4")
                        nc.tensor.matmul(PO4[0:4, :], WOUT[:, 0:4], h["H20"], start=True, stop=False)
                        nc.tensor.matmul(PO4[0:4, :], WOUT[:, 4:8], h["H21"], start=False, stop=True)
                        OT = ph.tile([4, NB], f32, tag="ot", name="OT")
                        nc.scalar.activation(OT, PO4[0:4, :], AF.Identity, bias=Bout)
                        nc.sync.dma_start(out=out_d[:, bs:bs + NB], in_=OT)
                        del state[t]

                    return [s1, s2, mk_h1(0), mk_h1(1), mk_h2(0), mk_h2(1), s7]

                # 3-deep software pipeline over tiles; head steps of tile
                # t-2 are spread through merged(t-1, t) so the head chain's
                # ACT latencies hide behind dense PE work
                load_x(0)
                load_x(1)
                merged(None, 0)
                load_late_consts()
                for t in range(1, NT):
                    if t + 1 < NT:
                        load_x(t + 1)
                    merged(t - 1, t)
                    if t >= 2:
                        for fn in head_steps(t - 2):
                            fn()
                for fn in head_steps(NT - 2):
                    fn()
                hs_last = head_steps(NT - 1)
                hs_last[1]()          # ownp: depends only on OWN, hide under ctx
                merged(NT - 1, None)
                hs_last[0]()
                for fn in hs_last[2:]:
                    fn()

    nc.compile()
    return nc


def _host_prep(inputs):
    """Build per-core input maps (numpy only)."""
    obs = np.ascontiguousarray(inputs["obs"], dtype=np.float32)
    w_own = np.asarray(inputs["w_own"], np.float32)
    w_int = np.asarray(inputs["w_int"], np.float32)
    wq = np.asarray(inputs["wq"], np.float32)
    wk = np.asarray(inputs["wk"], np.float32)
    wv = np.asarray(inputs["wv"], np.float32)
    v_att = np.asarray(inputs["v_att"], np.float32)
    w_attn = np.asarray(inputs["w_attn"], np.float32)
    w_ownp = np.asarray(inputs["w_ownp"], np.float32)
    w_h1 = np.asarray(inputs["w_h1"], np.float32)
    w_h2 = np.asarray(inputs["w_h2"], np.float32)
    w_out = np.asarray(inputs["w_out"], np.float32)

    def blockdiag(w):  # [H, D, D] -> [126, 126]
        out = np.zeros((TOT, TOT), np.float32)
        for h in range(H):
            out[h * D:(h + 1) * D, h * D:(h + 1) * D] = w[h]
        return out

    wia = np.zeros((126, 18 * 126), np.float32)
    for n in range(18):
        wia[7 * n:7 * n + 7, n * 126:(n + 1) * 126] = w_int
    wib = np.zeros((14, 2 * 126), np.float32)
    for n in range(2):
        wib[7 * n:7 * n + 7, n * 126:(n + 1) * 126] = w_int

    va32 = np.zeros((126, 32), np.float32)
    for h in range(H):
        va32[h * D:(h + 1) * D, h] = v_att[h]

    densel = np.zeros((128, 3), np.float32)
    for j in range(4):
        for h in range(H):
            densel[32 * j + h, h] = 1.0

    ebcsel = np.zeros((128, 4 * 126), np.float32)
    for j in range(4):
        for h in range(H):
            ebcsel[32 * j + h, j * 126 + h * D:(j * 126) + (h + 1) * D] = 1.0

    rbc = np.zeros((3, 126), np.float32)
    for h in range(H):
        rbc[h, h * D:(h + 1) * D] = 1.0

    wh1r = np.ascontiguousarray(
        w_h1.reshape(2, 128, HID).transpose(1, 0, 2).reshape(128, 512))
    wh2r = np.ascontiguousarray(
        w_h2.reshape(2, 128, HID).transpose(1, 0, 2).reshape(128, 512))
    woutr = np.ascontiguousarray(
        w_out.reshape(2, 128, NOUT).transpose(1, 0, 2).reshape(128, 8))

    params = {
        "wia": wia.astype(_bf16np), "wib": wib.astype(_bf16np), "wo": w_own.astype(_bf16np),
        "wqb": blockdiag(wq).astype(_bf16np), "wkb": blockdiag(wk).astype(_bf16np), "wvb": blockdiag(wv).astype(_bf16np),
        "va32": va32.astype(_bf16np), "densel": densel.astype(_bf16np), "ebcsel": ebcsel.astype(_bf16np), "rbc": rbc,
        "wat": w_attn, "wop": w_ownp.astype(_bf16np),
        "wh1r": wh1r, "wh2r": wh2r, "woutr": woutr,
        "bown": np.asarray(inputs["b_own"], np.float32).reshape(126, 1),
        "bint": np.asarray(inputs["b_int"], np.float32).reshape(126, 1),
        "bat": np.asarray(inputs["b_attn"], np.float32).reshape(128, 1),
        "bop": np.asarray(inputs["b_ownp"], np.float32).reshape(128, 1),
        "bh1": np.ascontiguousarray(
            np.asarray(inputs["b_h1"], np.float32).reshape(2, 128).T),
        "bh2": np.ascontiguousarray(
            np.asarray(inputs["b_h2"], np.float32).reshape(2, 128).T),
        "bout": np.asarray(inputs["b_out"], np.float32).reshape(4, 1),
    }

    in_maps = []
    for c in range(N_CORES):
        sl = obs[c * BC:(c + 1) * BC]                       # [BC, 147]
        xo = np.ascontiguousarray(sl[:, :OWN_DIM].T).astype(_bf16np)        # [7, BC]
        intr = sl[:, OWN_DIM:].reshape(BC, N_INTR, INT_DIM)  # [BC, 20, 7]
        intrT = intr.transpose(1, 2, 0)                     # [20, 7, BC]
        xa = np.ascontiguousarray(intrT[:18].reshape(126, BC)).astype(_bf16np)
        xb = np.ascontiguousarray(intrT[18:].reshape(14, BC)).astype(_bf16np)
        m = {"xo": xo, "xa": xa, "xb": xb}
        m.update(params)
        in_maps.append(m)
    return in_maps


def _get_nc():
    if "nc" not in _BUILT:
        _BUILT["nc"] = _build_nc()
    return _BUILT["nc"]


def run(inputs, trace=False):
    from concourse.bass_utils import run_bass_kernel_spmd
    nc = _get_nc()
    in_maps = _host_prep(inputs)
    res = run_bass_kernel_spmd(nc, in_maps, core_ids=list(range(N_CORES)),
                               trace=trace)
    outs = [res.results[c]["outT"] for c in range(N_CORES)]   # each [4, BC]
    full = np.concatenate(outs, axis=1).T                     # [B, 4]
    return np.ascontiguousarray(full, dtype=np.float32), res


def kernel(**inputs):
    out, _ = run(inputs, trace=False)
    return out


# revision 30
# speedup vs baseline: 1.0507x; 1.0094x over previous
"""Trainium2 Bass kernel for nn_AttentionSACModel (sparse_attention).

Data-parallel across 8 NeuronCores: obs sharded along batch, params replicated.
On-device layout keeps batch on the matmul free dim (activations stored
feature-major / transposed); all host<->device layout changes happen in numpy.
"""
import sys
import os

if "/opt/trn_rl_repo" not in sys.path:
    sys.path.insert(0, "/opt/trn_rl_repo")

import numpy as np
import ml_dtypes
_bf16np = ml_dtypes.bfloat16

OWN_DIM = 7
INT_DIM = 7
N_INTR = 20
H = 3
D = 42
TOT = H * D            # 126
ATTN = 128
HID = 256
NOUT = 4
B = 32768
N_CORES = 8
BC = B // N_CORES      # 4096 rows per core
NB = 512               # batch tile (matmul free dim)
NT = BC // NB          # 8 tiles per core
ALPHA = 0.2            # leaky relu slope

_BUILT = {}


def _build_nc():
    import concourse.bacc as bacc
    import concourse.bass as bass
    import concourse.tile as tile
    from concourse import mybir

    f32 = mybir.dt.float32
    f32r = mybir.dt.float32r
    bf16 = mybir.dt.bfloat16
    AF = mybir.ActivationFunctionType
    ALU = mybir.AluOpType
    AX = mybir.AxisListType

    nc = bacc.Bacc()

    # ---- DRAM I/O ----
    xo_d = nc.dram_tensor("xo", [OWN_DIM, BC], bf16, kind="ExternalInput")
    xa_d = nc.dram_tensor("xa", [126, BC], bf16, kind="ExternalInput")       # interactors 0..17, row 7n+f
    xb_d = nc.dram_tensor("xb", [14, BC], bf16, kind="ExternalInput")        # interactors 18,19
    wia_d = nc.dram_tensor("wia", [126, 18 * 126], bf16, kind="ExternalInput")  # padded int-embed lhsT, n<18
    wib_d = nc.dram_tensor("wib", [14, 2 * 126], bf16, kind="ExternalInput")    # n=18,19
    wo_d = nc.dram_tensor("wo", [7, 126], bf16, kind="ExternalInput")
    wq_d = nc.dram_tensor("wqb", [126, 126], bf16, kind="ExternalInput")
    wk_d = nc.dram_tensor("wkb", [126, 126], bf16, kind="ExternalInput")
    wv_d = nc.dram_tensor("wvb", [126, 126], bf16, kind="ExternalInput")
    va_d = nc.dram_tensor("va32", [126, 32], bf16, kind="ExternalInput")
    ds_d = nc.dram_tensor("densel", [128, 3], bf16, kind="ExternalInput")
    eb_d = nc.dram_tensor("ebcsel", [128, 4 * 126], bf16, kind="ExternalInput")
    rb_d = nc.dram_tensor("rbc", [3, 126], f32r, kind="ExternalInput")
    wat_d = nc.dram_tensor("wat", [126, 128], f32r, kind="ExternalInput")
    wop_d = nc.dram_tensor("wop", [126, 128], bf16, kind="ExternalInput")
    wh1_d = nc.dram_tensor("wh1r", [128, 512], f32r, kind="ExternalInput")   # [p, kc*256+m]
    wh2_d = nc.dram_tensor("wh2r", [128, 512], f32r, kind="ExternalInput")
    wout_d = nc.dram_tensor("woutr", [128, 8], f32r, kind="ExternalInput")   # [p, kc*4+m]
    bown_d = nc.dram_tensor("bown", [126, 1], f32, kind="ExternalInput")
    bint_d = nc.dram_tensor("bint", [126, 1], f32, kind="ExternalInput")
    bat_d = nc.dram_tensor("bat", [128, 1], f32, kind="ExternalInput")
    bop_d = nc.dram_tensor("bop", [128, 1], f32, kind="ExternalInput")
    bh1_d = nc.dram_tensor("bh1", [128, 2], f32, kind="ExternalInput")
    bh2_d = nc.dram_tensor("bh2", [128, 2], f32, kind="ExternalInput")
    bout_d = nc.dram_tensor("bout", [4, 1], f32, kind="ExternalInput")
    out_d = nc.dram_tensor("outT", [NOUT, BC], f32, kind="ExternalOutput")

    with tile.TileContext(nc) as tc:
        with tc.tile_pool(name="const", bufs=1) as cst, \
             tc.tile_pool(name="px", bufs=2) as px, \
             tc.tile_pool(name="pemb", bufs=5) as pemb, \
             tc.tile_pool(name="peng", bufs=4) as peng, \
             tc.tile_pool(name="pE", bufs=10) as pE, \
             tc.tile_pool(name="pv", bufs=2) as pv, \
             tc.tile_pool(name="pp", bufs=2) as pp, \
             tc.tile_pool(name="pn", bufs=8) as pn, \
             tc.tile_pool(name="ph", bufs=2) as ph, \
             tc.tile_pool(name="pz", bufs=2, space="PSUM") as ppz, \
             tc.tile_pool(name="pk", bufs=2, space="PSUM") as ppk, \
             tc.tile_pool(name="sm", bufs=3, space="PSUM") as small, \
             tc.tile_pool(name="pd", bufs=1, space="PSUM") as ppd:

            # ---- load constants ----
            WiA = cst.tile([126, 18 * 126], bf16)
            WiB = cst.tile([14, 2 * 126], bf16)
            Wo = cst.tile([7, 126], bf16)
            Wq = cst.tile([126, 126], bf16)
            Wk = cst.tile([126, 126], bf16)
            Wv = cst.tile([126, 126], bf16)
            Va = cst.tile([126, 32], bf16)
            Ds = cst.tile([128, 3], bf16)
            Eb = cst.tile([128, 4 * 126], bf16)
            Rb = cst.tile([3, 126], f32r)
            Wat = cst.tile([126, 128], f32r)
            Wop = cst.tile([126, 128], bf16)
            WH1 = cst.tile([128, 512], f32r)
            WH2 = cst.tile([128, 512], f32r)
            WOUT = cst.tile([128, 8], f32r)
            Bown = cst.tile([126, 1], f32)
            Bint = cst.tile([126, 1], f32)
            Bat = cst.tile([128, 1], f32)
            Bop = cst.tile([128, 1], f32)
            BH1 = cst.tile([128, 2], f32)
            BH2 = cst.tile([128, 2], f32)
            Bout = cst.tile([4, 1], f32)
            for t_sb, t_dr in [(WiA, wia_d), (Wo, wo_d), (Bown, bown_d),
                               (Bint, bint_d), (WiB, wib_d), (Wk, wk_d),
                               (Wq, wq_d), (Wv, wv_d), (Va, va_d)]:
                nc.sync.dma_start(out=t_sb, in_=t_dr[:, :])

            def load_late_consts():
                for t_sb, t_dr in [(Ds, ds_d), (Eb, eb_d), (Rb, rb_d),
                                   (Wat, wat_d), (Wop, wop_d), (WH1, wh1_d),
                                   (WH2, wh2_d), (WOUT, wout_d), (Bat, bat_d),
                                   (Bop, bop_d), (BH1, bh1_d), (BH2, bh2_d),
                                   (Bout, bout_d)]:
                    nc.scalar.dma_start(out=t_sb, in_=t_dr[:, :])

            with nc.allow_low_precision(reason="bf16/f32r intermediates; final accums are f32"):
                state = {}

                def load_x(t):
                    bs = t * NB
                    XO = px.tile([OWN_DIM, NB], bf16, tag="xo", name="XO")
                    XA = px.tile([126, NB], bf16, tag="xa", name="XA")
                    XB = px.tile([14, NB], bf16, tag="xb", name="XB")
                    nc.sync.dma_start(out=XO, in_=xo_d[:, bs:bs + NB])
                    nc.sync.dma_start(out=XA, in_=xa_d[:, bs:bs + NB])
                    nc.sync.dma_start(out=XB, in_=xb_d[:, bs:bs + NB])
                    state[t] = {"X": (XO, XA, XB)}

                def merged(t, tb):
                    """ctx phase of tile t (may be None) interleaved with
                    embed/attention phase of tile tb (may be None)."""
                    st = state.get(t)
                    if st is not None:
                        EGs = st["EGs"]
                        VA = st["VA"]
                        PD = ppd.tile([128, NB], f32, tag="pd", name="PD")
                        for g in range(5):
                            nc.tensor.matmul(PD[0:3, :], Ds, EGs[g],
                                             start=(g == 0), stop=(g == 4))
                        RD = ph.tile([3, NB], f32r, tag="rd", name="RD")
                        nc.vector.reciprocal(RD, PD[0:3, :])
                        PR = small.tile([128, NB], f32, tag="sm", name="PR")
                        nc.tensor.matmul(PR[0:126, :], Rb, RD)
                        TST = pp.tile([126, NB, N_INTR // 2], f32, tag="tst", name="TST")
                        PNs = []

                    if tb is not None:
                        XO, XA, XB = state[tb]["X"]
                        PO = small.tile([128, NB], f32, tag="sm", name="PO")
                        nc.tensor.matmul(PO[0:126, :], Wo, XO)
                        OWN = ph.tile([126, NB], bf16, tag="own", name="OWN", bufs=4)
                        nc.scalar.activation(OWN, PO[0:126, :], AF.Prelu, bias=Bown, alpha=ALPHA)
                        EGsb = []
                        VAb = pv.tile([126, N_INTR, NB], bf16, tag="va", name="VAb")
                        ZTs = {}
                        ENs = {}

                        def emit_z(n):
                            PZ = ppz.tile([126, NB], f32, tag="pz", name="PZ")
                            if n < 18:
                                nc.tensor.matmul(PZ, WiA[:, n * 126:(n + 1) * 126], XA)
                            else:
                                nc.tensor.matmul(PZ, WiB[:, (n - 18) * 126:(n - 17) * 126], XB)
                            ZT = pemb.tile([126, NB], bf16, tag="zt", name="ZT")
                            nc.scalar.activation(ZT, PZ, AF.Prelu, bias=Bint, alpha=ALPHA)
                            ZTs[n] = ZT

                        emit_z(0)
                        emit_z(1)
                        emit_z(2)
                        sc_pend = []

                        def emit_score(n, EN):
                            j = n % 4
                            if j == 0:
                                sc_pend.append(small.tile([128, NB], f32, tag="sm", name="PS"))
                            PSq = sc_pend[-1]
                            nc.tensor.matmul(PSq[32 * j:32 * (j + 1), :], Va, EN,
                                             tile_position=(0, 32 * j))
                            if j == 3:
                                EG = pE.tile([128, NB], bf16, tag="eg", name="EG")
                                nc.scalar.activation(EG, PSq, AF.Exp)
                                EGsb.append(EG)

                    for n in range(N_INTR):
                        if tb is not None:
                            ZT = ZTs.pop(n)
                            PK = ppk.tile([126, NB], f32, tag="pk", name="PK")
                            EN = peng.tile([126, NB], bf16, tag="en", name="EN")
                            nc.tensor.matmul(PK, Wk, ZT, start=True, stop=False)
                            nc.tensor.matmul(PK, Wq, OWN, start=False, stop=True)
                            nc.scalar.activation(EN, PK, AF.Tanh)

                            PV = small.tile([128, NB], f32, tag="sm", name="PV")
                            nc.tensor.matmul(PV[0:126, :], Wv, ZT)
                            nc.scalar.activation(VAb[:, n, :], PV[0:126, :], AF.Copy)

                            if n + 3 < N_INTR:
                                emit_z(n + 3)

                            j = n % 4
                            if j == 0:
                                PS = small.tile([128, NB], f32, tag="sm", name="PS")
                            nc.tensor.matmul(PS[32 * j:32 * (j + 1), :], Va, EN,
                                             tile_position=(0, 32 * j))
                            if j == 3:
                                EG = pE.tile([128, NB], bf16, tag="eg", name="EG")
                                nc.scalar.activation(EG, PS, AF.Exp)
                                EGsb.append(EG)

                        if st is not None and tb is None and n == 12:
                            CTXH0 = ph.tile([126, NB], f32, tag="ctxh", name="CTXH0")
                            nc.vector.tensor_reduce(CTXH0, TST[:, :, 0:5], axis=AX.X, op=ALU.add)
                            st["CTXH0"] = CTXH0
                        if st is not None:
                            g, j = n // 4, n % 4
                            PEb = small.tile([128, NB], f32, tag="sm", name="PEb")
                            nc.tensor.matmul(PEb[0:126, :], Eb[:, j * 126:(j + 1) * 126], EGs[g])
                            PN = pn.tile([126, NB], f32, tag="pn", name="PN")
                            nc.vector.tensor_tensor(out=PN, in0=PEb[0:126, :],
                                                    in1=VA[:, n, :], op=ALU.mult)
                            PNs.append(PN)
                            if n % 2 == 1:
                                nc.gpsimd.tensor_add(out=TST[:, :, n // 2],
                                                     in0=PNs[n - 1], in1=PNs[n])

                    if st is not None:
                        CTXU = ph.tile([126, NB], f32, tag="ctxu", name="CTXU")
                        if tb is None:
                            CTXH = st["CTXH0"]
                            CTXI = ph.tile([126, NB], f32, tag="ctxi", name="CTXI")
                            nc.vector.tensor_reduce(CTXI, TST[:, :, 5:10], axis=AX.X, op=ALU.add)
                            nc.vector.tensor_tensor(out=CTXU, in0=CTXH, in1=CTXI, op=ALU.add)
                        else:
                            nc.vector.tensor_reduce(CTXU, TST[:, :, :], axis=AX.X, op=ALU.add)
                        CTX = ph.tile([126, NB], f32r, tag="ctx", name="CTX")
                        nc.vector.tensor_tensor(out=CTX, in0=CTXU, in1=PR[0:126, :], op=ALU.mult)
                        st["CTX"] = CTX
                    if tb is not None:
                        state[tb].update({"OWN": OWN, "VA": VAb, "EGs": EGsb})

                def head_steps(t):
                    """head MLP + output for tile t, as interleavable steps"""
                    bs = t * NB
                    OWN = state[t]["OWN"]
                    h = {}

                    def s1():
                        PH1 = small.tile([128, NB], f32, tag="sm", name="PH1")
                        nc.tensor.matmul(PH1, Wat, state[t]["CTX"])
                        h["ATT"] = ph.tile([128, NB], f32r, tag="att", name="ATT")
                        nc.scalar.activation(h["ATT"], PH1, AF.Tanh, bias=Bat)

                    def s2():
                        PH2 = small.tile([128, NB], f32, tag="sm", name="PH2")
                        nc.tensor.matmul(PH2, Wop, OWN)
                        h["OWV"] = ph.tile([128, NB], f32r, tag="owv", name="OWV")
                        nc.scalar.activation(h["OWV"], PH2, AF.Tanh, bias=Bop)

                    def mk_h1(mh):
                        def s():
                            PHh = small.tile([128, NB], f32, tag="sm", name="PHh")
                            nc.tensor.matmul(PHh, WH1[:, mh * 128:(mh + 1) * 128], h["OWV"],
                                             start=True, stop=False)
                            nc.tensor.matmul(PHh, WH1[:, 256 + mh * 128:256 + (mh + 1) * 128],
                                             h["ATT"], start=False, stop=True)
                            h[f"H1{mh}"] = ph.tile([128, NB], f32r, tag=f"h1a{mh}", name="H1A")
                            nc.scalar.activation(h[f"H1{mh}"], PHh, AF.Prelu,
                                                 bias=BH1[:, mh:mh + 1], alpha=ALPHA)
                        return s

                    def mk_h2(mh):
                        def s():
                            PHh2 = small.tile([128, NB], f32, tag="sm", name="PHh2")
                            nc.tensor.matmul(PHh2, WH2[:, mh * 128:(mh + 1) * 128], h["H10"],
                                             start=True, stop=False)
                            nc.tensor.matmul(PHh2, WH2[:, 256 + mh * 128:256 + (mh + 1) * 128],
                                             h["H11"], start=False, stop=True)
                            h[f"H2{mh}"] = ph.tile([128, NB], f32r, tag=f"h2a{mh}", name="H2A")
                            nc.scalar.activation(h[f"H2{mh}"], PHh2, AF.Prelu,
                                                 bias=BH2[:, mh:mh + 1], alpha=ALPHA)
                        return s

                    def s7():
                        PO4 = small.tile([128, NB], f32, tag="sm", name="PO4")
                        nc.tensor.matmul(PO4[0:4, :], WOUT[:, 0:4], h["H20"], start=True, stop=False)
                        nc.tensor.matmul(PO4[0:4, :], WOUT[:, 4:8], h["H21"], start=False, stop=True)
                        OT = ph.tile([4, NB], f32, tag="ot", name="OT")
                        nc.scalar.activation(OT, PO4[0:4, :], AF.Identity, bias=Bout)
                        nc.sync.dma_start(out=out_d[:, bs:bs + NB], in_=OT)
                        del state[t]

                    return [s1, s2, mk_h1(0), mk_h1(1), mk_h2(0), mk_h2(1), s7]

                # 3-deep software pipeline over tiles; head steps of tile
                # t-2 are spread through merged(t-1, t) so the head chain's
                # ACT latencies hide behind dense PE work
                load_x(0)
                load_x(1)
                merged(None, 0)
                load_late_consts()
                for t in range(1, NT):
                    if t + 1 < NT:
                        load_x(t + 1)
                    merged(t - 1, t)
                    if t >= 2:
                        for fn in head_steps(t - 2):
                            fn()
                for fn in head_steps(NT - 2):
                    fn()
                hs_last = head_steps(NT - 1)
                hs_last[1]()          # ownp: depends only on OWN, hide under ctx
                merged(NT - 1, None)
                hs_last[0]()
                for fn in hs_last[2:]:
                    fn()

    nc.compile()
    return nc


def _host_prep(inputs):
    """Build per-core input maps (numpy only)."""
    obs = np.ascontiguousarray(inputs["obs"], dtype=np.float32)
    w_own = np.asarray(inputs["w_own"], np.float32)
    w_int = np.asarray(inputs["w_int"], np.float32)
    wq = np.asarray(inputs["wq"], np.float32)
    wk = np.asarray(inputs["wk"], np.float32)
    wv = np.asarray(inputs["wv"], np.float32)
    v_att = np.asarray(inputs["v_att"], np.float32)
    w_attn = np.asarray(inputs["w_attn"], np.float32)
    w_ownp = np.asarray(inputs["w_ownp"], np.float32)
    w_h1 = np.asarray(inputs["w_h1"], np.float32)
    w_h2 = np.asarray(inputs["w_h2"], np.float32)
    w_out = np.asarray(inputs["w_out"], np.float32)

    def blockdiag(w):  # [H, D, D] -> [126, 126]
        out = np.zeros((TOT, TOT), np.float32)
        for h in range(H):
            out[h * D:(h + 1) * D, h * D:(h + 1) * D] = w[h]
        return out

    wia = np.zeros((126, 18 * 126), np.float32)
    for n in range(18):
        wia[7 * n:7 * n + 7, n * 126:(n + 1) * 126] = w_int
    wib = np.zeros((14, 2 * 126), np.float32)
    for n in range(2):
        wib[7 * n:7 * n + 7, n * 126:(n + 1) * 126] = w_int

    va32 = np.zeros((126, 32), np.float32)
    for h in range(H):
        va32[h * D:(h + 1) * D, h] = v_att[h]

    densel = np.zeros((128, 3), np.float32)
    for j in range(4):
        for h in range(H):
            densel[32 * j + h, h] = 1.0

    ebcsel = np.zeros((128, 4 * 126), np.float32)
    for j in range(4):
        for h in range(H):
            ebcsel[32 * j + h, j * 126 + h * D:(j * 126) + (h + 1) * D] = 1.0

    rbc = np.zeros((3, 126), np.float32)
    for h in range(H):
        rbc[h, h * D:(h + 1) * D] = 1.0

    wh1r = np.ascontiguousarray(
        w_h1.reshape(2, 128, HID).transpose(1, 0, 2).reshape(128, 512))
    wh2r = np.ascontiguousarray(
        w_h2.reshape(2, 128, HID).transpose(1, 0, 2).reshape(128, 512))
    woutr = np.ascontiguousarray(
        w_out.reshape(2, 128, NOUT).transpose(1, 0, 2).reshape(128, 8))

    params = {
        "wia": wia.astype(_bf16np), "wib": wib.astype(_bf16np), "wo": w_own.astype(_bf16np),
        "wqb": blockdiag(wq).astype(_bf16np), "wkb": blockdiag(wk).astype(_bf16np), "wvb": blockdiag(wv).astype(_bf16np),
        "va32": va32.astype(_bf16np), "densel": densel.astype(_bf16np), "ebcsel": ebcsel.astype(_bf16np), "rbc": rbc,
        "wat": w_attn, "wop": w_ownp.astype(_bf16np),
        "wh1r": wh1r, "wh2r": wh2r, "woutr": woutr,
        "bown": np.asarray(inputs["b_own"], np.float32).reshape(126, 1),
        "bint": np.asarray(inputs["b_int"], np.float32).reshape(126, 1),
        "bat": np.asarray(inputs["b_attn"], np.float32).reshape(128, 1),
        "bop": np.asarray(inputs["b_ownp"], np.float32).reshape(128, 1),
        "bh1": np.ascontiguousarray(
            np.asarray(inputs["b_h1"], np.float32).reshape(2, 128).T),
        "bh2": np.ascontiguousarray(
            np.asarray(inputs["b_h2"], np.float32).reshape(2, 128).T),
        "bout": np.asarray(inputs["b_out"], np.float32).reshape(4, 1),
    }

    in_maps = []
    for c in range(N_CORES):
        sl = obs[c * BC:(c + 1) * BC]                       # [BC, 147]
        xo = np.ascontiguousarray(sl[:, :OWN_DIM].T).astype(_bf16np)        # [7, BC]
        intr = sl[:, OWN_DIM:].reshape(BC, N_INTR, INT_DIM)  # [BC, 20, 7]
        intrT = intr.transpose(1, 2, 0)                     # [20, 7, BC]
        xa = np.ascontiguousarray(intrT[:18].reshape(126, BC)).astype(_bf16np)
        xb = np.ascontiguousarray(intrT[18:].reshape(14, BC)).astype(_bf16np)
        m = {"xo": xo, "xa": xa, "xb": xb}
        m.update(params)
        in_maps.append(m)
    return in_maps


def _get_nc():
    if "nc" not in _BUILT:
        _BUILT["nc"] = _build_nc()
    return _BUILT["nc"]


def run(inputs, trace=False):
    from concourse.bass_utils import run_bass_kernel_spmd
    nc = _get_nc()
    in_maps = _host_prep(inputs)
    res = run_bass_kernel_spmd(nc, in_maps, core_ids=list(range(N_CORES)),
                               trace=trace)
    outs = [res.results[c]["outT"] for c in range(N_CORES)]   # each [4, BC]
    full = np.concatenate(outs, axis=1).T                     # [B, 4]
    return np.ascontiguousarray(full, dtype=np.float32), res


def kernel(**inputs):
    out, _ = run(inputs, trace=False)
    return out
